# revision 2
# baseline (speedup 1.0000x reference)
"""Causal single-head attention (B=4, S=2048, D=1024) on 8 Trainium2 NeuronCores.

Sharding: core = (batch, half). The two cores sharing a batch split the K/V
projection by SEQUENCE half (each core projects K^T and V for 1024 of the 2048
positions from its half of x), then exchange halves with pair-wise DRAM
AllGathers so each core ends up with the full K^T and V. This removes the
duplicated K/V projection a replicated design pays (4.3 GF/core).

Queries stay parity-sharded (chunks {0,3,4,7} on even cores, {1,2,5,6} on odd)
so causal attention work balances, and all per-core differences stay in input
data (masks, x slices) — the SPMD program is identical on every core.

Engine/queue plan (collectives block their engine, so they get gpsimd's tail):
  sync   : input DMAs (half), kT reloads, v reloads, output DMAs
  gpsimd : input DMAs (half), then cc1 (kT gather), cc2a/cc2b (v gather halves)
  scalar/vector : PSUM->SBUF copies + the bounce-buffer stage DMAs they feed

Device algorithm per core (bf16 operands, fp32 PSUM):
  kT_half[o, s_my]  = WkT.T @ x_half      -> bounce -> pair AllGather -> kT
  v_half[s_my, o]   = x_half.T @ WvT      -> bounce -> 2 AllGathers   -> v
  qT[o, q_my]       = (WqT/32).T @ xq     (softmax scale folded on host)
  per chunk L: scores_T[sk, sq] = kT_blk.T @ qT_chunk (SCHED=(4,8,12,16) slots)
               p = exp(scores_T) [* mask for the last 4 slots]
               attnV rows: sum_b p_b.T @ v_b with padded counts NK; den via
               ones-matmul; out = po/den -> fp16 -> DMA (host upcasts to fp32)
"""

import sys

if "/opt/trn_rl_repo" not in sys.path:
    sys.path.insert(0, "/opt/trn_rl_repo")

import numpy as np
import ml_dtypes

import concourse.mybir as mybir
import concourse.tile as tile
from concourse import bacc
from concourse.bass_utils import run_bass_kernel_spmd

bf16 = ml_dtypes.bfloat16

B, S, D = 4, 2048, 1024
H = S // 2           # per-core K/V sequence half
QC = 256             # attention query-chunk width
BLK = 128            # key block
SCHED = (4, 8, 12, 16)       # score-block slots per chunk (shared both parities)
MSK_BASE = (0, 4, 8, 12)     # slots b >= MSK_BASE[L] get a mask multiply
NMASK = 16
# attnV padded block counts per (chunk, sub-row): >= causal need of both parities
NK = ((3, 4), (7, 8), (11, 12), (15, 16))
DT8 = D // 128       # contraction tiles
N_CORES = 8
PAIRS = [[0, 1], [2, 3], [4, 5], [6, 7]]
DT_BF = mybir.dt.bfloat16
DT_F16 = mybir.dt.float16
DT_F32 = mybir.dt.float32

_NC_CACHE = {}


def _emit(tc, xh, xq, wq, wk, wv, msk, out):
    nc = tc.nc
    Exp = mybir.ActivationFunctionType.Exp

    ndma = [0]

    def _dma(out_, in_):
        eng = nc.sync if ndma[0] % 2 == 0 else nc.gpsimd
        ndma[0] += 1
        eng.dma_start(out=out_, in_=in_)

    with (
        tc.tile_pool(name="const", bufs=1) as constp,
        tc.tile_pool(name="kv", bufs=1) as kv,
        tc.tile_pool(name="dram", bufs=1, space="DRAM") as dram,
    ):
        ones = constp.tile([128, 1], DT_BF, tag="ones", name="ones")
        nc.vector.memset(ones, 1.0)

        kT_t = [kv.tile([128, S], DT_BF, tag=f"kT{i}", name=f"kT{i}")
                for i in range(DT8)]
        v_t = [kv.tile([128, D], DT_BF, tag=f"v{t}", name=f"v{t}")
               for t in range(S // 128)]
        qT_t = [kv.tile([128, D], DT_BF, tag=f"qT{i}", name=f"qT{i}")
                for i in range(DT8)]
        mall = kv.tile([128, NMASK * QC], DT_BF, tag="mall", name="mall")

        kb_in = dram.tile([D, H], DT_BF)
        kb_out = dram.tile([2 * D, H], DT_BF)
        vb_in = dram.tile([H, D], DT_BF)
        vb_out = dram.tile([S, D], DT_BF)

        with (
            tc.tile_pool(name="xw", bufs=1) as xw,
            tc.tile_pool(name="stg", bufs=4) as stg,
            tc.tile_pool(name="proj_ps", bufs=2, space="PSUM") as pps,
        ):
            wk_t = [xw.tile([128, D], DT_BF, tag=f"wk{i}", name=f"wk{i}")
                    for i in range(DT8)]
            xh_t = [
                [xw.tile([128, 512], DT_BF, tag=f"xh{i}_{sc}", name=f"xh{i}_{sc}")
                 for sc in range(2)]
                for i in range(DT8)
            ]
            wv_t = [xw.tile([128, D], DT_BF, tag=f"wv{i}", name=f"wv{i}")
                    for i in range(DT8)]
            wq_t = [xw.tile([128, D], DT_BF, tag=f"wq{i}", name=f"wq{i}")
                    for i in range(DT8)]
            xq_t = [xw.tile([128, D], DT_BF, tag=f"xq{i}", name=f"xq{i}")
                    for i in range(DT8)]

            # input DMAs in consumption order (kT proj first, masks last)
            for i in range(DT8):
                _dma(wk_t[i], wk[128 * i : 128 * (i + 1), :])
            for sc in range(2):
                for i in range(DT8):
                    _dma(xh_t[i][sc],
                         xh[128 * i : 128 * (i + 1), 512 * sc : 512 * (sc + 1)])
            for i in range(DT8):
                _dma(wv_t[i], wv[128 * i : 128 * (i + 1), :])
            for i in range(DT8):
                _dma(wq_t[i], wq[128 * i : 128 * (i + 1), :])
                _dma(xq_t[i], xq[128 * i : 128 * (i + 1), :])
            _dma(mall, msk)

            # kT-half projection: kT[o, s_my] = sum_i WkT[i,o] x[i,s_my]
            for ot in range(DT8):
                for sc in range(2):
                    ps = pps.tile([128, 512], DT_F32, tag="pps", name="pps")
                    for i in range(DT8):
                        nc.tensor.matmul(
                            ps,
                            lhsT=wk_t[i][:, 128 * ot : 128 * (ot + 1)],
                            rhs=xh_t[i][sc],
                            start=(i == 0),
                            stop=(i == DT8 - 1),
                        )
                    kstg = stg.tile([128, 512], DT_BF, tag="kstg", name="kstg")
                    if (ot + sc) % 2 == 0:
                        nc.scalar.copy(out=kstg, in_=ps)
                    else:
                        nc.vector.tensor_copy(out=kstg, in_=ps)
                    nc.scalar.dma_start(
                        out=kb_in[128 * ot : 128 * (ot + 1),
                                  512 * sc : 512 * (sc + 1)],
                        in_=kstg,
                    )
            nc.gpsimd.collective_compute(
                "AllGather",
                mybir.AluOpType.bypass,
                replica_groups=PAIRS,
                ins=[kb_in.opt()],
                outs=[kb_out.opt()],
            )
            # reload assembled kT: cols 0:H from rank0 block, H:S from rank1
            for ot in range(DT8):
                nc.sync.dma_start(
                    out=kT_t[ot][:, 0:H],
                    in_=kb_out[128 * ot : 128 * (ot + 1), :],
                )
                nc.sync.dma_start(
                    out=kT_t[ot][:, H:S],
                    in_=kb_out[D + 128 * ot : D + 128 * (ot + 1), :],
                )

            # v-half projection: v[s_my, o] = sum_i x[i,s_my] WvT[i,o]
            for st in range(H // 128):
                for oc in range(2):
                    ps = pps.tile([128, 512], DT_F32, tag="pps", name="pps")
                    for i in range(DT8):
                        nc.tensor.matmul(
                            ps,
                            lhsT=xh_t[i][st // 4][:, 128 * (st % 4) : 128 * (st % 4 + 1)],
                            rhs=wv_t[i][:, 512 * oc : 512 * (oc + 1)],
                            start=(i == 0),
                            stop=(i == DT8 - 1),
                        )
                    vstg = stg.tile([128, 512], DT_BF, tag="vstg", name="vstg")
                    if (st + oc) % 2 == 0:
                        nc.scalar.copy(out=vstg, in_=ps)
                    else:
                        nc.vector.tensor_copy(out=vstg, in_=ps)
                    nc.scalar.dma_start(
                        out=vb_in[128 * st : 128 * (st + 1),
                                  512 * oc : 512 * (oc + 1)],
                        in_=vstg,
                    )
            nc.gpsimd.collective_compute(
                "AllGather",
                mybir.AluOpType.bypass,
                replica_groups=PAIRS,
                ins=[vb_in.opt()],
                outs=[vb_out.opt()],
            )
            # vb_out rows are global s directly (rank0 = s-half 0)
            for t in range(S // 128):
                nc.sync.dma_start(
                    out=v_t[t], in_=vb_out[128 * t : 128 * (t + 1), :])

            # Q projection (Wq pre-scaled by 1/32 on host): qT[o, q_my]
            for ot in range(DT8):
                for qc in range(2):
                    ps = pps.tile([128, 512], DT_F32, tag="pps", name="pps")
                    for i in range(DT8):
                        nc.tensor.matmul(
                            ps,
                            lhsT=wq_t[i][:, 128 * ot : 128 * (ot + 1)],
                            rhs=xq_t[i][:, 512 * qc : 512 * (qc + 1)],
                            start=(i == 0),
                            stop=(i == DT8 - 1),
                        )
                    if (ot + qc) % 2 == 0:
                        nc.scalar.copy(
                            out=qT_t[ot][:, 512 * qc : 512 * (qc + 1)], in_=ps)
                    else:
                        nc.vector.tensor_copy(
                            out=qT_t[ot][:, 512 * qc : 512 * (qc + 1)], in_=ps)

        # ---- attention ----
        with (
            tc.tile_pool(name="attn_sb", bufs=1) as asb,
            tc.tile_pool(name="outs_sb", bufs=2) as osb,
            tc.tile_pool(name="score_ps", bufs=2, space="PSUM") as sps,
            tc.tile_pool(name="out_ps", bufs=2, space="PSUM") as ops,
            tc.tile_pool(name="den_ps", bufs=2, space="PSUM") as dps,
        ):
            for L in range(4):
                p_t = {}
                for b in range(SCHED[L]):
                    ps = sps.tile([128, QC], DT_F32, tag="sps", name="sps")
                    for i in range(DT8):
                        nc.tensor.matmul(
                            ps,
                            lhsT=kT_t[i][:, BLK * b : BLK * (b + 1)],
                            rhs=qT_t[i][:, QC * L : QC * (L + 1)],
                            start=(i == 0),
                            stop=(i == DT8 - 1),
                        )
                    es = asb.tile([128, QC], DT_BF, tag=f"es{L}_{b}",
                                  name=f"es{L}_{b}")
                    nc.scalar.activation(es, ps, Exp)
                    if b >= MSK_BASE[L]:
                        p = asb.tile([128, QC], DT_BF, tag=f"p{L}_{b}",
                                     name=f"p{L}_{b}")
                        s = 4 * L + b - MSK_BASE[L]
                        nc.vector.tensor_mul(
                            p, es, mall[:, QC * s : QC * (s + 1)])
                        p_t[b] = p
                    else:
                        p_t[b] = es

                for sr in range(2):
                    nblk = NK[L][sr]
                    po = ops.tile([128, D], DT_F32, tag="po", name="po")
                    pd = dps.tile([128, 1], DT_F32, tag="pd", name="pd")
                    for b in range(nblk):
                        pt = p_t[b][:, 128 * sr : 128 * (sr + 1)]
                        nc.tensor.matmul(
                            po[:, 0:512], lhsT=pt, rhs=v_t[b][:, 0:512],
                            start=(b == 0), stop=(b == nblk - 1),
                            skip_group_check=True,
                        )
                        nc.tensor.matmul(
                            po[:, 512:D], lhsT=pt, rhs=v_t[b][:, 512:D],
                            start=(b == 0), stop=(b == nblk - 1),
                            skip_group_check=True,
                        )
                        nc.tensor.matmul(
                            pd, lhsT=pt, rhs=ones,
                            start=(b == 0), stop=(b == nblk - 1),
                            skip_group_check=True,
                        )
                    r = osb.tile([128, 1], DT_F32, tag="r", name="r")
                    nc.vector.reciprocal(r, pd)
                    o = osb.tile([128, D], DT_F16, tag="osb", name="osb")
                    nc.vector.tensor_scalar_mul(o, po, r)
                    nc.sync.dma_start(
                        out=out[QC * L + 128 * sr : QC * L + 128 * (sr + 1), :],
                        in_=o,
                    )


def build_program():
    nc = bacc.Bacc(
        "TRN2",
        target_bir_lowering=False,
        debug=False,
        enable_asserts=False,
        num_devices=N_CORES,
    )
    xh = nc.dram_tensor("xh", [D, H], DT_BF, kind="ExternalInput").ap()
    xq = nc.dram_tensor("xq", [D, D], DT_BF, kind="ExternalInput").ap()
    wq = nc.dram_tensor("wq", [D, D], DT_BF, kind="ExternalInput").ap()
    wk = nc.dram_tensor("wk", [D, D], DT_BF, kind="ExternalInput").ap()
    wv = nc.dram_tensor("wv", [D, D], DT_BF, kind="ExternalInput").ap()
    msk = nc.dram_tensor("msk", [128, NMASK * QC], DT_BF, kind="ExternalInput").ap()
    out = nc.dram_tensor("out", [D, D], DT_F16, kind="ExternalOutput").ap()
    with tile.TileContext(nc) as tc:
        _emit(tc, xh, xq, wq, wk, wv, msk, out)
    nc.compile()
    return nc


def get_program():
    if "nc" not in _NC_CACHE:
        _NC_CACHE["nc"] = build_program()
    return _NC_CACHE["nc"]


def _chunks_for(core):
    """Per-core 256-wide query chunks, L-ordered to match SCHED=(4,8,12,16)."""
    return [0, 3, 4, 7] if core % 2 == 0 else [1, 2, 5, 6]


def _build_masks(chunks):
    """[128, 16*256] in {0,1}: slot s=4L+(b-MSK_BASE[L]) at cols 256s:
    allowed(sk=128b+p, sq=256*j+c) = sk <= sq; padding slots come out zero."""
    m = np.zeros((128, NMASK * QC), np.float32)
    p = np.arange(BLK)[:, None]
    c = np.arange(QC)[None, :]
    for L, j in enumerate(chunks):
        for b in range(MSK_BASE[L], SCHED[L]):
            s = 4 * L + b - MSK_BASE[L]
            m[:, QC * s : QC * (s + 1)] = BLK * b + p <= QC * j + c
    return m.astype(bf16)


def build_in_maps(x, Wq, Wk, Wv):
    wq = np.ascontiguousarray(Wq.T.astype(np.float32) / 32.0).astype(bf16)
    wk = np.ascontiguousarray(Wk.T).astype(bf16)
    wv = np.ascontiguousarray(Wv.T).astype(bf16)
    masks = {par: _build_masks(_chunks_for(par)) for par in (0, 1)}
    in_maps = []
    for core in range(N_CORES):
        b, h = core // 2, core % 2
        xTb = np.ascontiguousarray(x[b].T).astype(bf16)  # [D, S]
        xq = np.ascontiguousarray(
            np.concatenate(
                [xTb[:, QC * j : QC * (j + 1)] for j in _chunks_for(core)],
                axis=1,
            )
        )
        xh = np.ascontiguousarray(xTb[:, H * h : H * (h + 1)])
        in_maps.append(
            {"xh": xh, "xq": xq, "wq": wq, "wk": wk, "wv": wv,
             "msk": masks[core % 2]}
        )
    return in_maps


def assemble_output(results):
    out = np.zeros((B, S, D), np.float32)
    for core in range(N_CORES):
        b = core // 2
        res = np.asarray(results[core]["out"], dtype=np.float32)
        for L, j in enumerate(_chunks_for(core)):
            out[b, QC * j : QC * (j + 1)] = res[QC * L : QC * (L + 1)]
    return out


def kernel(x, Wq, Wk, Wv):
    x = np.asarray(x, np.float32)
    nc = get_program()
    in_maps = build_in_maps(x, np.asarray(Wq, np.float32),
                            np.asarray(Wk, np.float32), np.asarray(Wv, np.float32))
    res = run_bass_kernel_spmd(nc, in_maps, core_ids=list(range(N_CORES)))
    return assemble_output(res.results)
